# revision 15
# baseline (speedup 1.0000x reference)
"""GQA attention (qk-rmsnorm + partial RoPE) on 8 trn2 NeuronCores.

Sharding: sequence-parallel. B*S = 4096 rows split 8 ways (512 rows/core,
cores 0-3 = batch 0, cores 4-7 = batch 1). Each core projects q/k/v for its
rows (full head width, so the full-dim rmsnorm stays local), norms + ropes
its k rows, and AllGathers post-norm K / V across its 4-core batch group.
Attention + output projection are then fully local; the row-sharded outputs
are concatenated on the host.

Compute dtype bf16 (f32 psum accumulate, f32 softmax stats).

v6: broadcast-ones sum matmuls (no slow [1,512] stationaries, no DRAM-bounce
broadcasts), v/k AllGathers issued early and covered by the q projection,
KV gather DMAs on the gpsimd ring with 8KB-element layouts, lag-2
software-pipelined attention (PE never waits on exp), rope rotate-half via a
PE permutation matmul (no cross-partition shift DMAs), rmsnorm q-scale
interleaved per-head into attention, wo quarter-slab prefetch, deferred
ssq-table loads, bf16 output. The device is power-throttled (GPIO util limit
0.8125, ~92% active under dense load), so sustained matmul issue is ~262ns
per 512-col bf16 matmul; the schedule above keeps the PE within ~6%% of that
floor.
"""

import numpy as np
import ml_dtypes
from contextlib import ExitStack

import concourse.bass as bass
import concourse.tile as tile
from concourse import mybir, bacc
from concourse.bass_utils import run_bass_kernel_spmd
from concourse.masks import make_identity

B, S, H = 2, 2048, 4096
NQ, NK, D, RD = 32, 8, 128, 64
HALF = RD // 2
EPS = 1e-6
NCORES = 8
GRP = 4                      # cores per batch group
R = B * S // NCORES          # 512 rows per core
SCALE = D ** -0.5
BF16 = mybir.dt.bfloat16
F32 = mybir.dt.float32
NHC = H // 128               # 32 contraction chunks
KC = (GRP * R) // 128        # 16 k-row chunks per batch

_cache = {}


def _build():
    nc = bacc.Bacc("TRN2", target_bir_lowering=False, debug=False,
                   num_devices=NCORES)
    di = lambda n, s, d: nc.dram_tensor(n, s, d, kind="ExternalInput").ap()
    xT = di("xT", [H, R], BF16)
    wq = di("wq", [H, NQ * D], BF16)
    wk = di("wk", [H, NK * D], BF16)
    wv = di("wv", [H, NK * D], BF16)
    wo = di("wo", [NQ * D, H], BF16)
    cosT = di("cosT", [RD, R], BF16)
    sinTs = di("sinTs", [RD, R], BF16)        # rows 0:32 = -sinT, 32:64 = +sinT
    igq_bc = di("igq_bc", [128, NQ * 128], BF16)  # [d, c*128+j] = 1/gq[c*128+d]^2
    igk_bc = di("igk_bc", [128, NK * 128], BF16)
    prot = di("prot", [128, RD], BF16)            # rotate-half permutation
    out = nc.dram_tensor("out", [R, H], BF16, kind="ExternalOutput").ap()

    with tile.TileContext(nc) as tc:
        _emit(nc, tc, xT, wq, wk, wv, wo, cosT, sinTs, igq_bc, igk_bc, prot, out)
    nc.finalize()
    return nc


def _emit(nc, tc, xT, wq, wk, wv, wo, cosT, sinTs, igq_bc, igk_bc, prot, out):
    ctx = ExitStack()
    singles = ctx.enter_context(tc.tile_pool(name="singles", bufs=1))
    dram = ctx.enter_context(tc.tile_pool(name="dram", bufs=4, space="DRAM"))

    ident = singles.tile([128, 128], BF16)
    make_identity(nc, ident)
    eps_b = singles.tile([128, 1], F32)
    nc.vector.memset(eps_b, EPS)
    ones_bc = singles.tile([128, 128], BF16)
    nc.vector.memset(ones_bc, 1.0)
    prot_sb = singles.tile([128, RD], BF16)
    nc.sync.dma_start(out=prot_sb, in_=prot[:, :])
    rqp = ctx.enter_context(tc.tile_pool(name="rqp", bufs=1))

    attkv = ctx.enter_context(tc.tile_pool(name="attkv", bufs=1))
    kT_full = attkv.tile([128, GRP, NK, R], BF16)     # [d, j, h, r]
    qpool_cm = tc.tile_pool(name="qpool", bufs=1)
    qpool = qpool_cm.__enter__()
    qts = [qpool.tile([128, R], BF16, name=f"qh{c}", tag=f"qh{c}")
           for c in range(NQ)]

    # ---- AllGather bounce buffers ----
    agk_in = dram.tile([128, NK, R], BF16, tag="agki")
    agk_out = dram.tile([GRP, 128, NK, R], BF16, tag="agko")
    agv_in = dram.tile([128, R // 128, NK, D], BF16, tag="agvi")
    agv_out = dram.tile([GRP, 128, R // 128, NK, D], BF16, tag="agvo")
    rgroups = [[0, 1, 2, 3], [4, 5, 6, 7]]

    # =============== projections + norm + rope + AllGather ================
    with tc.tile_pool(name="kvtmp", bufs=1) as kvtmp, \
         tc.tile_pool(name="wpool", bufs=4) as wpool, \
         tc.tile_pool(name="sqpool", bufs=2) as sqpool, \
         tc.tile_pool(name="shp", bufs=2) as shp, \
         tc.tile_pool(name="pps", bufs=2, space="PSUM") as pps, \
         tc.tile_pool(name="projacc", bufs=1, space="PSUM") as projacc, \
         tc.tile_pool(name="rotps", bufs=2, space="PSUM") as rotps, \
         tc.tile_pool(name="tps", bufs=2, space="PSUM") as tps:

        cos_sb = kvtmp.tile([RD, R], BF16, tag="cos")
        nc.sync.dma_start(out=cos_sb, in_=cosT[:, :])
        sin_sb = kvtmp.tile([RD, R], BF16, tag="sin")
        nc.sync.dma_start(out=sin_sb, in_=sinTs[:, :])
        kT_all = kvtmp.tile([128, NK, R], BF16)   # roped/normed in place
        V_own = kvtmp.tile([128, R // 128, NK, D], BF16)   # [p, rc, h, d]
        xT_sb = kvtmp.tile([128, NHC, R], BF16)
        xTr = xT.rearrange("(j p) r -> p j r", p=128)
        for part in range(4):
            nc.sync.dma_start(out=xT_sb[:, part * 8:(part + 1) * 8, :],
                              in_=xTr[:, part * 8:(part + 1) * 8, :])

        pending = []

        def flush_ssq():
            while pending:
                ssq_tile, lhs, sq, first, last = pending.pop(0)
                nc.tensor.matmul(ssq_tile, lhs, sq,
                                 start=first, stop=last, skip_group_check=True)

        def proj_chunk(w_ap, c, ssq_lhs, ssq_tile, first, last, dst_ap):
            wsb = wpool.tile([128, NHC, 128], BF16, tag="wslab")
            nc.sync.dma_start(
                out=wsb,
                in_=w_ap[:, c * 128:(c + 1) * 128].rearrange("(j p) f -> p j f", p=128))
            ps = pps.tile([128, R], F32, tag="proj")
            for j in range(NHC):
                nc.tensor.matmul(ps, wsb[:, j, :], xT_sb[:, j, :],
                                 start=(j == 0), stop=(j == NHC - 1))
            nc.scalar.copy(out=dst_ap, in_=ps)
            flush_ssq()
            if ssq_tile is not None:
                sq = sqpool.tile([128, R], BF16, tag="sq")
                nc.vector.tensor_mul(sq, dst_ap, dst_ap)
                pending.append((ssq_tile, ssq_lhs[:, c, :], sq, first, last))

        def make_scale_bc(ssq_ps, inv_n, pool, tag):
            """rsqrt(ssq/n + eps) on the broadcast [128, R] ssq, no DMA."""
            rt = pool.tile([128, R], F32, tag=tag + "s")
            nc.scalar.activation(out=rt, in_=ssq_ps,
                                 func=mybir.ActivationFunctionType.Sqrt,
                                 bias=eps_b, scale=inv_n)
            bc = pool.tile([128, R], BF16, tag=tag)
            with nc.allow_low_precision(reason="rms scale fine in bf16"):
                nc.vector.reciprocal(out=bc, in_=rt)
            return bc

        def rope_rot_inplace(buf_ap):
            """buf[0:RD] = rope-rotate(buf[0:RD]), in place (no norm scale).
            Rotate-half via PE permutation matmul (no cross-partition DMA)."""
            shps = rotps.tile([RD, R], F32, tag="rot")
            nc.tensor.matmul(shps, prot_sb, buf_ap, start=True, stop=True,
                             skip_group_check=True)
            sh = shp.tile([RD, R], BF16, tag="shift")
            nc.vector.tensor_mul(sh, shps, sin_sb)
            t2 = shp.tile([RD, R], BF16, tag="tcos")
            nc.vector.tensor_mul(t2, buf_ap[0:RD, :], cos_sb)
            nc.vector.tensor_add(buf_ap[0:RD, :], t2, sh)

        def rope_norm_inplace(buf_ap, bc):
            rope_rot_inplace(buf_ap)
            nc.vector.tensor_mul(buf_ap, buf_ap, bc)

        # ---- v projection (+transpose) first, AllGather v early ----
        for c in range(NK):
            vt = sqpool.tile([128, R], BF16, tag="vtchunk")
            proj_chunk(wv, c, None, None, False, False, vt[:, :])
            for rc in range(R // 128):
                tp = tps.tile([128, 128], BF16, tag="vtp")
                nc.tensor.transpose(tp, vt[:, rc * 128:(rc + 1) * 128], ident)
                nc.scalar.copy(out=V_own[:, rc, c, :], in_=tp)
        nc.gpsimd.dma_start(out=agv_in[:], in_=V_own)
        nc.gpsimd.collective_compute(
            "AllGather", mybir.AluOpType.bypass, replica_groups=rgroups,
            ins=[agv_in.opt()], outs=[agv_out.opt()])

        # ---- k projection + norm + rope, AllGather k ----
        igq_sb = kvtmp.tile([128, NQ, 128], BF16, tag="igq")
        nc.sync.dma_start(out=igq_sb, in_=igq_bc.rearrange("p (c j) -> p c j", j=128))
        igk_sb = kvtmp.tile([128, NK, 128], BF16, tag="igk")
        nc.sync.dma_start(out=igk_sb, in_=igk_bc.rearrange("p (c j) -> p c j", j=128))
        ssqk = projacc.tile([128, R], F32, tag="ssqk")
        for c in range(NK):
            proj_chunk(wk, c, igk_sb, ssqk, c == 0, c == NK - 1, kT_all[:, c, :])
        flush_ssq()
        rk_bc = make_scale_bc(ssqk, 1.0 / (NK * D), kvtmp, "rk")
        for c in range(NK):
            rope_norm_inplace(kT_all[:, c, :], rk_bc)
        nc.gpsimd.dma_start(out=agk_in[:], in_=kT_all)
        nc.gpsimd.collective_compute(
            "AllGather", mybir.AluOpType.bypass, replica_groups=rgroups,
            ins=[agk_in.opt()], outs=[agk_out.opt()])
        # gather K on the gpsimd ring so it cannot block weight prefetch
        for j in range(GRP):
            nc.gpsimd.dma_start(out=kT_full[:, j, :, :], in_=agk_out[j])

        # ---- q projection + norm + rope (covers the AllGathers) ----
        ssqq = projacc.tile([128, R], F32, tag="ssqq")
        for c in range(NQ):
            proj_chunk(wq, c, igq_sb, ssqq, c == 0, c == NQ - 1, qts[c][:, :])
            rope_rot_inplace(qts[c][:, :])
        flush_ssq()
        rq_bc = make_scale_bc(ssqq, 1.0 / (NQ * D), rqp, "rq")

    # ========================== attention =================================
    aots = [singles.tile([128, R], BF16, name=f"ao{c}", tag=f"ao{c}")
            for c in range(NQ)]
    wop_cm = tc.tile_pool(name="wop", bufs=2)
    wop = wop_cm.__enter__()
    NWQ = 8                        # wo quarter-slab head count

    def load_wo_quarter(hp, quarter):
        wosb = wop.tile([128, NWQ, 512], BF16, tag="wo", name=f"wo{hp}_{quarter}")
        nc.sync.dma_start(
            out=wosb,
            in_=wo[quarter * NWQ * 128:(quarter + 1) * NWQ * 128,
                   hp * 512:(hp + 1) * 512].rearrange(
                "(j p) f -> p j f", p=128))
        return wosb

    LAG = 2
    with tc.tile_pool(name="vfp", bufs=1) as vfp, \
         tc.tile_pool(name="expp", bufs=3) as expp, \
         tc.tile_pool(name="recp", bufs=2) as recp, \
         tc.tile_pool(name="attps", bufs=2, space="PSUM") as attps, \
         tc.tile_pool(name="zps", bufs=2, space="PSUM") as zps, \
         tc.tile_pool(name="pvps", bufs=2, space="PSUM") as pvps:
        V_full = vfp.tile([128, KC, NK, D], BF16)     # [p, kc, h, d]
        for j in range(GRP):
            nc.gpsimd.dma_start(
                out=V_full[:, j * (R // 128):(j + 1) * (R // 128), :, :],
                in_=agv_out[j])

        def kT_chunk(kvh, kc):
            return kT_full[:, kc // 4, kvh, (kc % 4) * 128:(kc % 4) * 128 + 128]

        wo_q0 = load_wo_quarter(0, 0)   # prefetched during attention

        exs, zs, pvs = {}, {}, {}

        def sum_pv(qh):
            kvh = qh // GRP
            ex, zbc_ps, out_ps = exs[qh], zs[qh], pvs[qh]
            for kk in range(KC):
                nc.tensor.matmul(zbc_ps, ones_bc, ex[:, kk, :],
                                 start=(kk == 0), stop=(kk == KC - 1),
                                 skip_group_check=True)
            for kk in range(KC):
                nc.tensor.matmul(out_ps, V_full[:, kk, kvh, :], ex[:, kk, :],
                                 start=(kk == 0), stop=(kk == KC - 1),
                                 skip_group_check=True)

        def rec_aot(qh):
            rec_bc = recp.tile([128, R], F32, tag="rec_bc")
            nc.vector.reciprocal(out=rec_bc, in_=zs[qh])
            nc.vector.tensor_mul(aots[qh][:, :], pvs[qh], rec_bc)
            del exs[qh], zs[qh], pvs[qh]

        for qh in range(NQ):
            kvh = qh // GRP
            exs[qh] = expp.tile([128, KC, R], BF16, tag="expT", name=f"ex{qh}")
            ex = exs[qh]
            nc.vector.tensor_mul(qts[qh][:, :], qts[qh][:, :], rq_bc)
            lg = qh - LAG
            if lg >= 0:
                zs[lg] = zps.tile([128, R], F32, tag="zbc", name=f"z{lg}")
                pvs[lg] = pvps.tile([128, R], F32, tag="pv", name=f"pv{lg}")
            for t in range(KC // 2):
                sps2 = attps.tile([128, 2, R], F32, tag="sps2")
                nc.tensor.matmul(sps2[:, 0, :], kT_chunk(kvh, 2 * t),
                                 qts[qh][:, :], start=True, stop=True,
                                 skip_group_check=True)
                nc.tensor.matmul(sps2[:, 1, :], kT_chunk(kvh, 2 * t + 1),
                                 qts[qh][:, :], start=True, stop=True,
                                 skip_group_check=True)
                nc.scalar.activation(out=ex[:, 2 * t:2 * t + 2, :], in_=sps2,
                                     func=mybir.ActivationFunctionType.Exp,
                                     scale=SCALE)
            if lg >= 0:
                sum_pv(lg)
                rec_aot(lg)
        for lg in range(NQ - LAG, NQ):
            zs[lg] = zps.tile([128, R], F32, tag="zbc", name=f"zt{lg}")
            pvs[lg] = pvps.tile([128, R], F32, tag="pv", name=f"pvt{lg}")
            sum_pv(lg)
            rec_aot(lg)

    # ======================= output projection ============================
    with tc.tile_pool(name="odr", bufs=4) as odr, \
         tc.tile_pool(name="ops", bufs=1, space="PSUM") as ops:
        NHP = H // 512
        NQUART = NQ // NWQ
        for hp in range(NHP):
            quarters = {}
            for quarter in range(2):
                if hp == 0 and quarter == 0:
                    quarters[0] = wo_q0
                else:
                    quarters[quarter] = load_wo_quarter(hp, quarter)
            pos = [ops.tile([128, 512], F32, tag=f"ops{i}", name=f"po{hp}_{i}")
                   for i in range(4)]
            for h in range(NQ):
                if h % NWQ == 0 and h // NWQ + 2 < NQUART:
                    q_next = h // NWQ + 2
                    quarters[q_next] = load_wo_quarter(hp, q_next)
                wosb = quarters[h // NWQ]
                for rc in range(R // 128):
                    nc.tensor.matmul(
                        pos[rc],
                        aots[h][:, rc * 128:(rc + 1) * 128],
                        wosb[:, h % NWQ, :],
                        start=(h == 0), stop=(h == NQ - 1),
                        skip_group_check=True)
            for rc in range(R // 128):
                osb = odr.tile([128, 512], BF16, tag="odr")
                nc.scalar.copy(out=osb, in_=pos[rc])
                nc.sync.dma_start(
                    out=out[rc * 128:(rc + 1) * 128, hp * 512:(hp + 1) * 512],
                    in_=osb)
    wop_cm.__exit__(None, None, None)
    qpool_cm.__exit__(None, None, None)
    ctx.close()


def _get_nc():
    if "nc" not in _cache:
        _cache["nc"] = _build()
    return _cache["nc"]


def kernel(x, cos, sin, wq, wk, wv, wo, gq, gk):
    bf = ml_dtypes.bfloat16
    x = np.asarray(x, np.float32)
    cos = np.asarray(cos, np.float32)
    sin = np.asarray(sin, np.float32)
    gq = np.asarray(gq, np.float32)
    gk = np.asarray(gk, np.float32)
    wqp = (np.asarray(wq, np.float32) * gq[None, :]).astype(bf)
    wkp = (np.asarray(wk, np.float32) * gk[None, :]).astype(bf)
    wv_b = np.asarray(wv, np.float32).astype(bf)
    wo_b = np.asarray(wo, np.float32).astype(bf)
    igq = np.where(gq == 0, 0, 1.0 / np.maximum(gq * gq, 1e-30)).astype(np.float32)
    igk = np.where(gk == 0, 0, 1.0 / np.maximum(gk * gk, 1e-30)).astype(np.float32)
    # broadcast ssq weights: [d, c*128 + j] = ig[c*128 + d] for all j
    igq_bc = np.repeat(igq.reshape(NQ, 128).T[:, :, None], 128, axis=2)
    igq_bc = igq_bc.reshape(128, NQ * 128).astype(bf)
    igk_bc = np.repeat(igk.reshape(NK, 128).T[:, :, None], 128, axis=2)
    igk_bc = igk_bc.reshape(128, NK * 128).astype(bf)
    protm = np.zeros((128, RD), np.float32)
    for p in range(HALF):
        protm[p + HALF, p] = 1.0          # out[p] = in[p+32] for p < 32
    for p in range(HALF, RD):
        protm[p - HALF, p] = 1.0          # out[p] = in[p-32] for 32 <= p < 64
    protm = protm.astype(bf)

    x2 = x.reshape(B * S, H)
    in_maps = []
    for c in range(NCORES):
        p0 = (c % GRP) * R
        sinT = sin[p0:p0 + R].T.astype(np.float32)        # [RD, R]
        sinTs = np.concatenate([-sinT[:HALF], sinT[HALF:]], 0)
        in_maps.append({
            "xT": np.ascontiguousarray(x2[c * R:(c + 1) * R].T).astype(bf),
            "wq": wqp, "wk": wkp, "wv": wv_b, "wo": wo_b,
            "cosT": np.ascontiguousarray(cos[p0:p0 + R].T).astype(bf),
            "sinTs": np.ascontiguousarray(sinTs).astype(bf),
            "igq_bc": igq_bc, "igk_bc": igk_bc, "prot": protm,
        })
    nc = _get_nc()
    import os
    kw = {}
    if os.environ.get("KERNEL_TRACE"):
        kw = dict(trace=True, tmpdir=os.environ.get("KERNEL_TRACE_DIR") or None)
    res = run_bass_kernel_spmd(nc, in_maps, core_ids=list(range(NCORES)), **kw)
    kernel.last_exec_time_ns = res.exec_time_ns
    outp = np.concatenate([res.results[c]["out"] for c in range(NCORES)], 0)
    return outp.reshape(B, S, H).astype(np.float32)


# revision 21
# speedup vs baseline: 1.0329x; 1.0329x over previous
"""GQA attention (qk-rmsnorm + partial RoPE) on 8 trn2 NeuronCores.

Sharding: sequence-parallel. B*S = 4096 rows split 8 ways (512 rows/core,
cores 0-3 = batch 0, cores 4-7 = batch 1). Each core projects q/k/v for its
rows (full head width, so the full-dim rmsnorm stays local), norms + ropes
its k rows, and AllGathers post-norm K / V across its 4-core batch group.
Attention + output projection are then fully local; the row-sharded outputs
are concatenated on the host.

Compute dtype bf16 (f32 psum accumulate, f32 softmax stats).

v6: broadcast-ones sum matmuls (no slow [1,512] stationaries, no DRAM-bounce
broadcasts), v/k AllGathers issued early and covered by the q projection,
KV gather DMAs on the gpsimd ring with 8KB-element layouts, lag-2
software-pipelined attention (PE never waits on exp), rope rotate-half via a
PE permutation matmul (no cross-partition shift DMAs), rmsnorm q-scale
interleaved per-head into attention, wo quarter-slab prefetch, deferred
ssq-table loads, bf16 output. The device is power-throttled (GPIO util limit
0.8125, ~92% active under dense load), so sustained matmul issue is ~262ns
per 512-col bf16 matmul; the schedule above keeps the PE within ~6%% of that
floor.
"""

import numpy as np
import ml_dtypes
from contextlib import ExitStack

import concourse.bass as bass
import concourse.tile as tile
from concourse import mybir, bacc
from concourse.bass_utils import run_bass_kernel_spmd
from concourse.masks import make_identity

B, S, H = 2, 2048, 4096
NQ, NK, D, RD = 32, 8, 128, 64
HALF = RD // 2
EPS = 1e-6
NCORES = 8
GRP = 4                      # cores per batch group
R = B * S // NCORES          # 512 rows per core
SCALE = D ** -0.5
BF16 = mybir.dt.bfloat16
F32 = mybir.dt.float32
NHC = H // 128               # 32 contraction chunks
KC = (GRP * R) // 128        # 16 k-row chunks per batch

_cache = {}


def _build():
    nc = bacc.Bacc("TRN2", target_bir_lowering=False, debug=False,
                   num_devices=NCORES)
    di = lambda n, s, d: nc.dram_tensor(n, s, d, kind="ExternalInput").ap()
    xT = di("xT", [H, R], BF16)
    wq = di("wq", [H, NQ * D], BF16)
    wk = di("wk", [H, NK * D], BF16)
    wv = di("wv", [H, NK * D], BF16)
    wo = di("wo", [NQ * D, H], BF16)
    cosT = di("cosT", [RD, R], BF16)
    sinTs = di("sinTs", [RD, R], BF16)        # rows 0:32 = -sinT, 32:64 = +sinT
    igq_bc = di("igq_bc", [128, NQ * 128], BF16)  # [d, c*128+j] = 1/gq[c*128+d]^2
    igk_bc = di("igk_bc", [128, NK * 128], BF16)
    prot = di("prot", [128, RD], BF16)            # rotate-half permutation
    out = nc.dram_tensor("out", [R, H], BF16, kind="ExternalOutput").ap()

    with tile.TileContext(nc) as tc:
        _emit(nc, tc, xT, wq, wk, wv, wo, cosT, sinTs, igq_bc, igk_bc, prot, out)
    nc.finalize()
    return nc


def _emit(nc, tc, xT, wq, wk, wv, wo, cosT, sinTs, igq_bc, igk_bc, prot, out):
    ctx = ExitStack()
    singles = ctx.enter_context(tc.tile_pool(name="singles", bufs=1))
    dram = ctx.enter_context(tc.tile_pool(name="dram", bufs=4, space="DRAM"))

    ident = singles.tile([128, 128], BF16)
    make_identity(nc, ident)
    eps_b = singles.tile([128, 1], F32)
    nc.vector.memset(eps_b, EPS)
    ones_bc = singles.tile([128, 128], BF16)
    nc.vector.memset(ones_bc, 1.0)
    prot_sb = singles.tile([128, RD], BF16)
    nc.sync.dma_start(out=prot_sb, in_=prot[:, :])
    rqp = ctx.enter_context(tc.tile_pool(name="rqp", bufs=1))

    attkv = ctx.enter_context(tc.tile_pool(name="attkv", bufs=1))
    kT_full = attkv.tile([128, GRP, NK, R], BF16)     # [d, j, h, r]
    qpool_cm = tc.tile_pool(name="qpool", bufs=1)
    qpool = qpool_cm.__enter__()
    qts = [qpool.tile([128, R], BF16, name=f"qh{c}", tag=f"qh{c}")
           for c in range(NQ)]

    # ---- AllGather bounce buffers ----
    agk_in = dram.tile([128, NK, R], BF16, tag="agki")
    agk_out = dram.tile([GRP, 128, NK, R], BF16, tag="agko")
    agv_in = dram.tile([128, R // 128, NK, D], BF16, tag="agvi")
    agv_out = dram.tile([GRP, 128, R // 128, NK, D], BF16, tag="agvo")
    rgroups = [[0, 1, 2, 3], [4, 5, 6, 7]]

    # =============== projections + norm + rope + AllGather ================
    with tc.tile_pool(name="kvtmp", bufs=1) as kvtmp, \
         tc.tile_pool(name="wpool", bufs=4) as wpool, \
         tc.tile_pool(name="sqpool", bufs=2) as sqpool, \
         tc.tile_pool(name="shp", bufs=2) as shp, \
         tc.tile_pool(name="pps", bufs=2, space="PSUM") as pps, \
         tc.tile_pool(name="projacc", bufs=1, space="PSUM") as projacc, \
         tc.tile_pool(name="rotps", bufs=2, space="PSUM") as rotps, \
         tc.tile_pool(name="tps", bufs=2, space="PSUM") as tps:

        cos_sb = kvtmp.tile([RD, R], BF16, tag="cos")
        nc.sync.dma_start(out=cos_sb, in_=cosT[:, :])
        sin_sb = kvtmp.tile([RD, R], BF16, tag="sin")
        nc.sync.dma_start(out=sin_sb, in_=sinTs[:, :])
        kT_all = kvtmp.tile([128, NK, R], BF16)   # roped/normed in place
        V_own = kvtmp.tile([128, R // 128, NK, D], BF16)   # [p, rc, h, d]
        xT_sb = kvtmp.tile([128, NHC, R], BF16)
        xTr = xT.rearrange("(j p) r -> p j r", p=128)
        for part in range(4):
            nc.sync.dma_start(out=xT_sb[:, part * 8:(part + 1) * 8, :],
                              in_=xTr[:, part * 8:(part + 1) * 8, :])

        pending = []

        def flush_ssq():
            while pending:
                ssq_tile, lhs, sq, first, last = pending.pop(0)
                nc.tensor.matmul(ssq_tile, lhs, sq,
                                 start=first, stop=last, skip_group_check=True)

        def proj_chunk(w_ap, c, ssq_lhs, ssq_tile, first, last, dst_ap):
            wsb = wpool.tile([128, NHC, 128], BF16, tag="wslab")
            nc.sync.dma_start(
                out=wsb,
                in_=w_ap[:, c * 128:(c + 1) * 128].rearrange("(j p) f -> p j f", p=128))
            ps = pps.tile([128, R], F32, tag="proj")
            for j in range(NHC):
                nc.tensor.matmul(ps, wsb[:, j, :], xT_sb[:, j, :],
                                 start=(j == 0), stop=(j == NHC - 1))
            nc.scalar.copy(out=dst_ap, in_=ps)
            flush_ssq()
            if ssq_tile is not None:
                sq = sqpool.tile([128, R], BF16, tag="sq")
                nc.vector.tensor_mul(sq, dst_ap, dst_ap)
                pending.append((ssq_tile, ssq_lhs[:, c, :], sq, first, last))

        def make_scale_bc(ssq_ps, inv_n, pool, tag):
            """rsqrt(ssq/n + eps) on the broadcast [128, R] ssq, no DMA."""
            rt = pool.tile([128, R], F32, tag=tag + "s")
            nc.scalar.activation(out=rt, in_=ssq_ps,
                                 func=mybir.ActivationFunctionType.Sqrt,
                                 bias=eps_b, scale=inv_n)
            bc = pool.tile([128, R], BF16, tag=tag)
            with nc.allow_low_precision(reason="rms scale fine in bf16"):
                nc.vector.reciprocal(out=bc, in_=rt)
            return bc

        def rope_rot_inplace(buf_ap):
            """buf[0:RD] = rope-rotate(buf[0:RD]), in place (no norm scale).
            Rotate-half via PE permutation matmul (no cross-partition DMA)."""
            shps = rotps.tile([RD, R], F32, tag="rot")
            nc.tensor.matmul(shps, prot_sb, buf_ap, start=True, stop=True,
                             skip_group_check=True)
            sh = shp.tile([RD, R], BF16, tag="shift")
            nc.vector.tensor_mul(sh, shps, sin_sb)
            t2 = shp.tile([RD, R], BF16, tag="tcos")
            nc.vector.tensor_mul(t2, buf_ap[0:RD, :], cos_sb)
            nc.vector.tensor_add(buf_ap[0:RD, :], t2, sh)

        def rope_norm_inplace(buf_ap, bc):
            rope_rot_inplace(buf_ap)
            nc.vector.tensor_mul(buf_ap, buf_ap, bc)

        # ---- v projection (+transpose) first, AllGather v early ----
        for c in range(NK):
            vt = sqpool.tile([128, R], BF16, tag="vtchunk")
            proj_chunk(wv, c, None, None, False, False, vt[:, :])
            for rc in range(R // 128):
                tp = tps.tile([128, 128], BF16, tag="vtp")
                nc.tensor.transpose(tp, vt[:, rc * 128:(rc + 1) * 128], ident)
                nc.scalar.copy(out=V_own[:, rc, c, :], in_=tp)
        nc.gpsimd.dma_start(out=agv_in[:], in_=V_own)
        nc.gpsimd.collective_compute(
            "AllGather", mybir.AluOpType.bypass, replica_groups=rgroups,
            ins=[agv_in.opt()], outs=[agv_out.opt()])

        # ---- k projection + norm + rope, AllGather k ----
        igq_sb = kvtmp.tile([128, NQ, 128], BF16, tag="igq")
        nc.sync.dma_start(out=igq_sb, in_=igq_bc.rearrange("p (c j) -> p c j", j=128))
        igk_sb = kvtmp.tile([128, NK, 128], BF16, tag="igk")
        nc.sync.dma_start(out=igk_sb, in_=igk_bc.rearrange("p (c j) -> p c j", j=128))
        ssqk = projacc.tile([128, R], F32, tag="ssqk")
        for c in range(NK):
            proj_chunk(wk, c, igk_sb, ssqk, c == 0, c == NK - 1, kT_all[:, c, :])
        flush_ssq()
        rk_bc = make_scale_bc(ssqk, 1.0 / (NK * D), kvtmp, "rk")
        for c in range(NK):
            rope_norm_inplace(kT_all[:, c, :], rk_bc)
        nc.gpsimd.dma_start(out=agk_in[:], in_=kT_all)
        nc.gpsimd.collective_compute(
            "AllGather", mybir.AluOpType.bypass, replica_groups=rgroups,
            ins=[agk_in.opt()], outs=[agk_out.opt()])
        # gather K on the gpsimd ring so it cannot block weight prefetch
        for j in range(GRP):
            nc.gpsimd.dma_start(out=kT_full[:, j, :, :], in_=agk_out[j])

        # ---- q projection + norm + rope (covers the AllGathers) ----
        ssqq = projacc.tile([128, R], F32, tag="ssqq")
        for c in range(NQ):
            proj_chunk(wq, c, igq_sb, ssqq, c == 0, c == NQ - 1, qts[c][:, :])
            rope_rot_inplace(qts[c][:, :])
        flush_ssq()
        rq_bc = make_scale_bc(ssqq, 1.0 / (NQ * D), rqp, "rq")

    # ========================== attention =================================
    aots = [singles.tile([128, R], BF16, name=f"ao{c}", tag=f"ao{c}")
            for c in range(NQ)]
    wop_cm = tc.tile_pool(name="wop", bufs=2)
    wop = wop_cm.__enter__()
    NWQ = 8                        # wo quarter-slab head count

    def load_wo_quarter(hp, quarter):
        wosb = wop.tile([128, NWQ, 512], BF16, tag="wo", name=f"wo{hp}_{quarter}")
        nc.sync.dma_start(
            out=wosb,
            in_=wo[quarter * NWQ * 128:(quarter + 1) * NWQ * 128,
                   hp * 512:(hp + 1) * 512].rearrange(
                "(j p) f -> p j f", p=128))
        return wosb

    LAG = 2
    with tc.tile_pool(name="vfp", bufs=1) as vfp, \
         tc.tile_pool(name="expp", bufs=3) as expp, \
         tc.tile_pool(name="recp", bufs=2) as recp, \
         tc.tile_pool(name="attps", bufs=2, space="PSUM") as attps, \
         tc.tile_pool(name="zps", bufs=2, space="PSUM") as zps, \
         tc.tile_pool(name="pvps", bufs=2, space="PSUM") as pvps:
        V_full = vfp.tile([128, KC, NK, D], BF16)     # [p, kc, h, d]
        for j in range(GRP):
            nc.gpsimd.dma_start(
                out=V_full[:, j * (R // 128):(j + 1) * (R // 128), :, :],
                in_=agv_out[j])

        def kT_chunk(kvh, kc):
            return kT_full[:, kc // 4, kvh, (kc % 4) * 128:(kc % 4) * 128 + 128]

        wo_q0 = load_wo_quarter(0, 0)   # prefetched during attention

        exs, zs, pvs = {}, {}, {}

        def sum_pv(qh):
            kvh = qh // GRP
            ex, zbc_ps, out_ps = exs[qh], zs[qh], pvs[qh]
            for kk in range(KC):
                nc.tensor.matmul(zbc_ps, ones_bc, ex[:, kk, :],
                                 start=(kk == 0), stop=(kk == KC - 1),
                                 skip_group_check=True)
            for kk in range(KC):
                nc.tensor.matmul(out_ps, V_full[:, kk, kvh, :], ex[:, kk, :],
                                 start=(kk == 0), stop=(kk == KC - 1),
                                 skip_group_check=True)

        def rec_aot(qh):
            rec_bc = recp.tile([128, R], F32, tag="rec_bc")
            nc.vector.reciprocal(out=rec_bc, in_=zs[qh])
            nc.vector.tensor_mul(aots[qh][:, :], pvs[qh], rec_bc)
            del exs[qh], zs[qh], pvs[qh]

        for qh in range(NQ):
            kvh = qh // GRP
            exs[qh] = expp.tile([128, KC, R], BF16, tag="expT", name=f"ex{qh}")
            ex = exs[qh]
            nc.vector.tensor_mul(qts[qh][:, :], qts[qh][:, :], rq_bc)
            lg = qh - LAG
            if lg >= 0:
                zs[lg] = zps.tile([128, R], F32, tag="zbc", name=f"z{lg}")
                pvs[lg] = pvps.tile([128, R], F32, tag="pv", name=f"pv{lg}")
            for t in range(KC // 2):
                sps2 = attps.tile([128, 2, R], F32, tag="sps2")
                nc.tensor.matmul(sps2[:, 0, :], kT_chunk(kvh, 2 * t),
                                 qts[qh][:, :], start=True, stop=True,
                                 skip_group_check=True)
                nc.tensor.matmul(sps2[:, 1, :], kT_chunk(kvh, 2 * t + 1),
                                 qts[qh][:, :], start=True, stop=True,
                                 skip_group_check=True)
                nc.scalar.activation(out=ex[:, 2 * t:2 * t + 2, :], in_=sps2,
                                     func=mybir.ActivationFunctionType.Exp,
                                     scale=SCALE)
            if lg >= 0:
                sum_pv(lg)
                rec_aot(lg)
        for lg in range(NQ - LAG, NQ):
            zs[lg] = zps.tile([128, R], F32, tag="zbc", name=f"zt{lg}")
            pvs[lg] = pvps.tile([128, R], F32, tag="pv", name=f"pvt{lg}")
            sum_pv(lg)
            rec_aot(lg)

    # ======================= output projection ============================
    with tc.tile_pool(name="odr", bufs=4) as odr, \
         tc.tile_pool(name="ops", bufs=1, space="PSUM") as ops:
        NHP = H // 512
        NQUART = NQ // NWQ
        for hp in range(NHP):
            quarters = {}
            for quarter in range(2):
                if hp == 0 and quarter == 0:
                    quarters[0] = wo_q0
                else:
                    quarters[quarter] = load_wo_quarter(hp, quarter)
            pos = [ops.tile([128, 512], F32, tag=f"ops{i}", name=f"po{hp}_{i}")
                   for i in range(4)]
            for h in range(NQ):
                if h % NWQ == 0 and h // NWQ + 2 < NQUART:
                    q_next = h // NWQ + 2
                    quarters[q_next] = load_wo_quarter(hp, q_next)
                wosb = quarters[h // NWQ]
                for rc in range(R // 128):
                    nc.tensor.matmul(
                        pos[rc],
                        aots[h][:, rc * 128:(rc + 1) * 128],
                        wosb[:, h % NWQ, :],
                        start=(h == 0), stop=(h == NQ - 1),
                        skip_group_check=True)
            for rc in range(R // 128):
                osb = odr.tile([128, 512], BF16, tag="odr")
                nc.scalar.copy(out=osb, in_=pos[rc])
                nc.sync.dma_start(
                    out=out[rc * 128:(rc + 1) * 128, hp * 512:(hp + 1) * 512],
                    in_=osb)
    wop_cm.__exit__(None, None, None)
    qpool_cm.__exit__(None, None, None)
    ctx.close()


def _get_nc():
    if "nc" not in _cache:
        _cache["nc"] = _build()
    return _cache["nc"]


def kernel(x, cos, sin, wq, wk, wv, wo, gq, gk):
    bf = ml_dtypes.bfloat16
    x = np.asarray(x, np.float32)
    cos = np.asarray(cos, np.float32)
    sin = np.asarray(sin, np.float32)
    gq = np.asarray(gq, np.float32)
    gk = np.asarray(gk, np.float32)
    wqp = (np.asarray(wq, np.float32) * gq[None, :]).astype(bf)
    wkp = (np.asarray(wk, np.float32) * gk[None, :]).astype(bf)
    wv_b = np.asarray(wv, np.float32).astype(bf)
    wo_b = np.asarray(wo, np.float32).astype(bf)
    igq = np.where(gq == 0, 0, 1.0 / np.maximum(gq * gq, 1e-30)).astype(np.float32)
    igk = np.where(gk == 0, 0, 1.0 / np.maximum(gk * gk, 1e-30)).astype(np.float32)
    # broadcast ssq weights: [d, c*128 + j] = ig[c*128 + d] for all j
    igq_bc = np.repeat(igq.reshape(NQ, 128).T[:, :, None], 128, axis=2)
    igq_bc = igq_bc.reshape(128, NQ * 128).astype(bf)
    igk_bc = np.repeat(igk.reshape(NK, 128).T[:, :, None], 128, axis=2)
    igk_bc = igk_bc.reshape(128, NK * 128).astype(bf)
    protm = np.zeros((128, RD), np.float32)
    for p in range(HALF):
        protm[p + HALF, p] = 1.0          # out[p] = in[p+32] for p < 32
    for p in range(HALF, RD):
        protm[p - HALF, p] = 1.0          # out[p] = in[p-32] for 32 <= p < 64
    protm = protm.astype(bf)

    x2 = x.reshape(B * S, H)
    in_maps = []
    for c in range(NCORES):
        p0 = (c % GRP) * R
        sinT = sin[p0:p0 + R].T.astype(np.float32)        # [RD, R]
        sinTs = np.concatenate([-sinT[:HALF], sinT[HALF:]], 0)
        in_maps.append({
            "xT": np.ascontiguousarray(x2[c * R:(c + 1) * R].T).astype(bf),
            "wq": wqp, "wk": wkp, "wv": wv_b, "wo": wo_b,
            "cosT": np.ascontiguousarray(cos[p0:p0 + R].T).astype(bf),
            "sinTs": np.ascontiguousarray(sinTs).astype(bf),
            "igq_bc": igq_bc, "igk_bc": igk_bc, "prot": protm,
        })
    nc = _get_nc()
    import os
    kw = {}
    if os.environ.get("KERNEL_TRACE"):
        kw = dict(trace=True, tmpdir=os.environ.get("KERNEL_TRACE_DIR") or None)
    res = run_bass_kernel_spmd(nc, in_maps, core_ids=list(range(NCORES)), **kw)
    kernel.last_exec_time_ns = res.exec_time_ns
    outp = np.concatenate([res.results[c]["out"] for c in range(NCORES)], 0)
    return outp.reshape(B, S, H).astype(np.float32)
